# revision 1
# baseline (speedup 1.0000x reference)
"""ChameleonAttention Trainium2 kernel.

Full-input contract: kernel(**inputs) with the complete tensors; internally
shards tensor-parallel across 8 NeuronCores by attention head (4 heads/core):
  - w_qkv rows + q/k norm params sharded by head
  - w_o columns sharded by head, partial outputs summed on host (all-reduce)

Per-core dataflow (matmuls in fp32r, 1 cycle/row):
  P1: qkvT[r, t] = w_qkv_c @ hidden^T via PE-transposed tiles; K split in two
      halves (wT half kept SBUF-resident), accumulated into a DRAM scratch
      with an accumulate-DMA on the second half.
  P2: per head: LayerNorm (PE ones-matmul stats + K=1 broadcast matmuls),
      neox RoPE (device-computed sin/cos with Cody-Waite range reduction),
      causal attention streamed over key tiles in S^T layout: softmax
      denominator via ones-matmul (no max subtraction; scores are O(1)),
      E = exp(scale*S^T) straight out of PSUM, P@V accumulated transposed.
  P3: o_proj: out[t, :] = attnT^T @ w_o_c^T with PE-transposed w_o tiles.
"""
import sys

sys.path.insert(0, "/opt/trn_rl_repo")

import numpy as np

import concourse.bass as bass
import concourse.mybir as mybir
import concourse.tile as tile
from concourse import bacc
from concourse.bass_utils import run_bass_kernel_spmd
from concourse.masks import make_identity, make_upper_triangular

P = 128
T = 2048
HID = 4096
D = 128
H_PER_CORE = 4
R = 3 * H_PER_CORE  # qkv row-tiles per core (4 q + 4 k + 4 v)
KH = HID // 2  # contraction half
TC = 256  # P1 t-chunk
THETA = 10000.0
EPS = 1e-5
SCALE = D ** -0.5
TWO_PI = 6.283185307179586
C_HI = float(np.float32(6.28125))
C_LO = TWO_PI - C_HI

f32 = mybir.dt.float32
f32r = mybir.dt.float32r
i32 = mybir.dt.int32
AF = mybir.ActivationFunctionType
ALU = mybir.AluOpType

_NC_CACHE = {}


def build_nc(n_iters: int = 1, dump: bool = False, phases: str = "123"):
    nc = bacc.Bacc(None, target_bir_lowering=False, debug=False)

    hidden = nc.dram_tensor("hidden", (T, HID), f32, kind="ExternalInput")
    wq = nc.dram_tensor("wq", (R * P, HID), f32, kind="ExternalInput")
    wo = nc.dram_tensor("wo", (HID, H_PER_CORE * D), f32, kind="ExternalInput")
    pos = nc.dram_tensor("pos", (T,), i32, kind="ExternalInput")
    qnw = nc.dram_tensor("qnw", (H_PER_CORE, D), f32, kind="ExternalInput")
    qnb = nc.dram_tensor("qnb", (H_PER_CORE, D), f32, kind="ExternalInput")
    knw = nc.dram_tensor("knw", (H_PER_CORE, D), f32, kind="ExternalInput")
    knb = nc.dram_tensor("knb", (H_PER_CORE, D), f32, kind="ExternalInput")
    out = nc.dram_tensor("out", (T, HID), f32, kind="ExternalOutput")

    if "1" in phases:
        qkvT = nc.dram_tensor("qkvT_scr", (R, P, T), f32)  # internal scratch
    else:
        qkvT = nc.dram_tensor("qkvT_scr", (R, P, T), f32, kind="ExternalInput")
    if "3" in phases and "2" not in phases:
        attnT_in = nc.dram_tensor("attnT_in", (P, H_PER_CORE, T), f32,
                                  kind="ExternalInput")
    if dump:
        d_qkvT = nc.dram_tensor("d_qkvT", (R, P, T), f32, kind="ExternalOutput")
        d_x = nc.dram_tensor("d_x", (8, P, T), f32, kind="ExternalOutput")
        d_attnT = nc.dram_tensor("d_attnT", (P, H_PER_CORE, T), f32, kind="ExternalOutput")

    with tile.TileContext(nc) as tc:
        with tc.tile_pool(name="const", bufs=1) as const:
            # --- constants ---
            ident = const.tile([P, P], f32r)
            triu = const.tile([P, P], f32r)
            ones_c = const.tile([P, 1], f32r)
            ones_r = const.tile([1, P], f32r)
            zeros_r = const.tile([P, 3 * P], f32r)
            eps8 = const.tile([8, 1], f32)
            nc.vector.memset(eps8[:], EPS)
            with tc.tile_pool(name="cstage", bufs=1) as cstage:
                ident_f = cstage.tile([P, P], f32)
                make_identity(nc, ident_f[:])
                nc.vector.tensor_copy(ident[:], ident_f[:])
                triu_f = cstage.tile([P, P], f32)
                make_upper_triangular(nc, triu_f[:], val=1.0, diag=True)
                nc.vector.tensor_copy(triu[:], triu_f[:])
                ones_f = cstage.tile([P, 1], f32)
                nc.vector.memset(ones_f[:], 1.0)
                nc.vector.tensor_copy(ones_c[:], ones_f[:])
                ones_rf = cstage.tile([1, P], f32)
                nc.vector.memset(ones_rf[:], 1.0)
                nc.vector.tensor_copy(ones_r[:], ones_rf[:])
                zeros_f = cstage.tile([P, 3 * P], f32)
                nc.vector.memset(zeros_f[:], 0.0)
                nc.vector.tensor_copy(zeros_r[:], zeros_f[:])

            # norm params as [128, 1] per-partition columns; cols = head
            nwq = const.tile([P, H_PER_CORE], f32)
            nbq = const.tile([P, H_PER_CORE], f32)
            nwk = const.tile([P, H_PER_CORE], f32)
            nbk = const.tile([P, H_PER_CORE], f32)
            for h in range(H_PER_CORE):
                for dst, src in ((nwq, qnw), (nbq, qnb), (nwk, knw), (nbk, knb)):
                    nc.sync.dma_start(
                        dst[:, h : h + 1], src[h : h + 1, :].rearrange("a b -> b a")
                    )

            # --- rope tables: c128 = [cos; cos], s128 = [-sin; sin] ---
            c128 = const.tile([P, T], f32)
            s128 = const.tile([P, T], f32)
            with tc.tile_pool(name="rope_tmp", bufs=1) as rtmp:
                tp_i = rtmp.tile([1, T], i32)
                nc.sync.dma_start(tp_i[:], pos[None, :])
                tp_f = rtmp.tile([1, T], f32)
                nc.vector.tensor_copy(tp_f[:], tp_i[:])
                posb = rtmp.tile([64, T], f32)
                nc.gpsimd.partition_broadcast(posb[:], tp_f[:], channels=64)
                jj = rtmp.tile([64, 1], f32)
                nc.gpsimd.iota(jj[:], pattern=[[1, 1]], base=0, channel_multiplier=1,
                               allow_small_or_imprecise_dtypes=True)
                invf = rtmp.tile([64, 1], f32)
                nc.scalar.activation(invf[:], jj[:], AF.Exp,
                                     scale=-float(np.log(THETA)) / 64.0)
                freqs = rtmp.tile([64, T], f32)
                nc.vector.tensor_scalar_mul(freqs[:], posb[:], invf[:])

                def reduced_sin(dst_ap, src_ap, sign):
                    # dst = sin(sign * reduce(src)), reduce(x) = x - 2pi*round(x/2pi)
                    q = rtmp.tile([64, T], f32, tag="rs_q")
                    nc.vector.tensor_scalar_mul(q[:], src_ap, 1.0 / TWO_PI)
                    n_i = rtmp.tile([64, T], i32, tag="rs_n")
                    nc.vector.tensor_copy(n_i[:], q[:])  # round-to-nearest
                    n_f = rtmp.tile([64, T], f32, tag="rs_nf")
                    nc.vector.tensor_copy(n_f[:], n_i[:])
                    r0 = rtmp.tile([64, T], f32, tag="rs_r0")
                    nc.vector.scalar_tensor_tensor(
                        out=r0[:], in0=n_f[:], scalar=-C_HI, in1=src_ap,
                        op0=ALU.mult, op1=ALU.add)
                    r1 = rtmp.tile([64, T], f32, tag="rs_r1")
                    nc.vector.scalar_tensor_tensor(
                        out=r1[:], in0=n_f[:], scalar=-C_LO, in1=r0[:],
                        op0=ALU.mult, op1=ALU.add)
                    nc.scalar.activation(dst_ap, r1[:], AF.Sin, scale=sign)

                reduced_sin(s128[0:64, :], freqs[:], -1.0)
                reduced_sin(s128[64:P, :], freqs[:], 1.0)
                fr2 = rtmp.tile([64, T], f32)
                nc.vector.tensor_scalar_add(fr2[:], freqs[:], np.pi / 2)
                reduced_sin(c128[0:64, :], fr2[:], 1.0)
                nc.vector.tensor_copy(c128[64:P, :], c128[0:64, :])

            def _phases(_iv=None):
                if "1" in phases:
                    # ---------------- P1: qkvT = wq @ hidden^T ----------------
                    with (
                        tc.tile_pool(name="p1_wld", bufs=2) as p_wld,
                        tc.tile_pool(name="p1_wT", bufs=1) as p_wT,
                        tc.tile_pool(name="p1_hld", bufs=2) as p_hld,
                        tc.tile_pool(name="p1_hT", bufs=2) as p_hT,
                        tc.tile_pool(name="p1_ev", bufs=2) as p_ev,
                        tc.tile_pool(name="p1_tps", bufs=2, space="PSUM") as p_tps,
                        tc.tile_pool(name="p1_pack", bufs=2, space="PSUM") as p_pack,
                    ):
                        NKK = KH // P  # 16 k-tiles per half
                        for kh in range(2):
                            k0 = kh * KH
                            wTs = [p_wT.tile([P, NKK, P], f32r, tag=f"wT{rt}", name=f"wT{rt}")
                                   for rt in range(R)]
                            for rt in range(R):
                                wld = p_wld.tile([P, KH], f32r, tag="wld")
                                nc.sync.dma_start(
                                    wld[:], wq[rt * P : (rt + 1) * P, k0 : k0 + KH].bitcast(f32r)
                                )
                                for kg in range(NKK // 4):
                                    pw = p_tps.tile([P, 4, P], f32r, tag="tps")
                                    for j in range(4):
                                        kk = kg * 4 + j
                                        nc.tensor.transpose(
                                            pw[:, j, :],
                                            wld[:, kk * P : (kk + 1) * P],
                                            ident[:],
                                        )
                                    nc.any.tensor_copy(
                                        wTs[rt][:, kg * 4 : kg * 4 + 4, :], pw[:])
                            for tc_i in range(T // TC):
                                t0 = tc_i * TC
                                hT = p_hT.tile([P, NKK, TC], f32r, tag="hT")
                                for tt in range(TC // P):
                                    hld = p_hld.tile([P, KH], f32r, tag="hld")
                                    nc.sync.dma_start(
                                        hld[:],
                                        hidden[t0 + tt * P : t0 + (tt + 1) * P,
                                               k0 : k0 + KH].bitcast(f32r),
                                    )
                                    for kg in range(NKK // 4):
                                        ph = p_tps.tile([P, 4, P], f32r, tag="tps")
                                        for j in range(4):
                                            kk = kg * 4 + j
                                            nc.tensor.transpose(
                                                ph[:, j, :],
                                                hld[:, kk * P : (kk + 1) * P],
                                                ident[:],
                                            )
                                        nc.any.tensor_copy(
                                            hT[:, kg * 4 : kg * 4 + 4, tt * P : (tt + 1) * P],
                                            ph[:],
                                        )
                                for rg in range(2):
                                    pack = p_pack.tile([P, 6, TC], f32, tag="pack")
                                    # rr outer / kk inner: each slot's accumulation chain
                                    # completes before the next one's start=True clears
                                    # the shared bank's has_written bits
                                    for rr in range(6):
                                        rt = rg * 6 + rr
                                        for kk in range(NKK):
                                            nc.tensor.matmul(
                                                pack[:, rr, :],
                                                wTs[rt][:, kk, :],
                                                hT[:, kk, :],
                                                start=(kk == 0),
                                                stop=(kk == NKK - 1),
                                            )
                                    ev = p_ev.tile([P, 6, TC], f32, tag="ev")
                                    nc.any.tensor_copy(ev[:, 0:3, :], pack[:, 0:3, :])
                                    nc.any.tensor_copy(ev[:, 3:6, :], pack[:, 3:6, :])
                                    dst = qkvT[rg * 6 : rg * 6 + 6, :, t0 : t0 + TC]
                                    dst = dst.rearrange("r p t -> p r t")
                                    if kh == 0:
                                        nc.sync.dma_start(dst, ev[:])
                                    else:
                                        nc.gpsimd.dma_start(dst, ev[:], accum_op=ALU.add)

                    if dump:
                        for rt in range(R):
                            nc.sync.dma_start(d_qkvT[rt], qkvT[rt])

                # ---------------- P2 + P3 ----------------
                with (
                    tc.tile_pool(name="p2_attnT", bufs=1) as p_attnT,
                    tc.tile_pool(name="ps_misc", bufs=2, space="PSUM") as ps_misc,
                ):
                    attnT = p_attnT.tile([P, H_PER_CORE, T], f32r, tag="attnT")

                    if "2" in phases:
                        with (
                            tc.tile_pool(name="p2_x", bufs=1) as p_x,
                            tc.tile_pool(name="p2_sq", bufs=1) as p_sq,
                            tc.tile_pool(name="p2_st", bufs=1) as p_st,
                            tc.tile_pool(name="p2_v", bufs=2) as p_v,
                            tc.tile_pool(name="p2_vsb", bufs=1) as p_vsb,
                            tc.tile_pool(name="p2_E", bufs=3) as p_E,
                            tc.tile_pool(name="p2_sw", bufs=1) as p_sw,
                            tc.tile_pool(name="p2_rec", bufs=2) as p_rec,
                        ):
                            xs = []
                            with tc.tile_pool(name="ps_stats", bufs=2, space="PSUM") as ps_stats:
                                for hh in range(2 * H_PER_CORE):
                                    x = p_x.tile([P, T], f32r, tag=f"x{hh}")
                                    xs.append(x)
                                    nc.sync.dma_start(x[:], qkvT[hh, :, :].bitcast(f32r))
                                    sq = p_sq.tile([P, T], f32r, tag="sq")
                                    nc.scalar.activation(sq[:], x[:], AF.Square)
                                    w_col = (nwq if hh < 4 else nwk)[:, (hh % 4) : (hh % 4) + 1]
                                    b_col = (nbq if hh < 4 else nbk)[:, (hh % 4) : (hh % 4) + 1]
                                    for c4 in range(4):
                                        cs = slice(c4 * 512, (c4 + 1) * 512)
                                        s1_ps = ps_stats.tile([1, 512], f32, tag="s1")
                                        nc.tensor.matmul(s1_ps[:], ones_c[:], x[:, cs],
                                                         start=True, stop=True)
                                        s2_ps = ps_stats.tile([1, 512], f32, tag="s2")
                                        nc.tensor.matmul(s2_ps[:], ones_c[:], sq[:, cs],
                                                         start=True, stop=True)
                                        mu_sb = p_st.tile([1, 512], f32r, tag="mu")
                                        nc.scalar.activation(mu_sb[:], s1_ps[:], AF.Copy,
                                                             scale=1.0 / D)
                                        musq = p_st.tile([1, 512], f32, tag="musq")
                                        nc.scalar.activation(musq[:], mu_sb[:], AF.Square)
                                        varv = p_st.tile([1, 512], f32, tag="varv")
                                        nc.vector.scalar_tensor_tensor(
                                            out=varv[:], in0=s2_ps[:], scalar=1.0 / D,
                                            in1=musq[:], op0=ALU.mult, op1=ALU.subtract)
                                        stdv = p_st.tile([1, 512], f32, tag="stdv")
                                        nc.scalar.activation(stdv[:], varv[:], AF.Sqrt,
                                                             bias=eps8[0:1, :])
                                        rstd_sb = p_st.tile([1, 512], f32r, tag="rstd")
                                        with nc.allow_low_precision(reason="f32r LN rstd"):
                                            nc.vector.reciprocal(rstd_sb[:], stdv[:])
                                        mu_b = ps_misc.tile([P, 512], f32, tag="misc")
                                        nc.tensor.matmul(mu_b[:], ones_r[:], mu_sb[:],
                                                         start=True, stop=True)
                                        rs_b = ps_misc.tile([P, 512], f32, tag="misc")
                                        nc.tensor.matmul(rs_b[:], ones_r[:], rstd_sb[:],
                                                         start=True, stop=True)
                                        nc.vector.tensor_sub(x[:, cs], x[:, cs], mu_b[:])
                                        nc.vector.tensor_mul(x[:, cs], x[:, cs], rs_b[:])
                                        nc.vector.tensor_scalar(
                                            out=x[:, cs], in0=x[:, cs],
                                            scalar1=w_col, scalar2=b_col,
                                            op0=ALU.mult, op1=ALU.add)

                            # rope per head
                            for hh in range(2 * H_PER_CORE):
                                x = xs[hh]
                                sw = p_sw.tile([P, T], f32, tag="sw")
                                nc.gpsimd.tensor_copy(sw[0:64, :], x[64:P, :])
                                nc.gpsimd.tensor_copy(sw[64:P, :], x[0:64, :])
                                nc.gpsimd.tensor_mul(sw[:], sw[:], s128[:])
                                nc.vector.tensor_mul(x[:], x[:], c128[:])
                                nc.vector.tensor_add(x[:], x[:], sw[:])

                            if dump:
                                for hh in range(2 * H_PER_CORE):
                                    nc.sync.dma_start(d_x[hh], xs[hh][:].bitcast(f32))

                            # attention per q head
                            for h in range(H_PER_CORE):
                                xq = xs[h]
                                xk = xs[4 + h]
                                xv = p_v.tile([P, T], f32r, tag="xv")
                                nc.sync.dma_start(xv[:], qkvT[8 + h, :, :].bitcast(f32r))
                                v_sb = p_vsb.tile([P, T // P, P], f32r, tag="v_sb")
                                for g in range(4):
                                    pv = ps_misc.tile([P, 4, P], f32r, tag="misc")
                                    for j in range(4):
                                        i = g * 4 + j
                                        nc.tensor.transpose(
                                            pv[:, j, :], xv[:, i * P : (i + 1) * P], ident[:]
                                        )
                                    nc.any.tensor_copy(v_sb[:, g * 4 : g * 4 + 4, :], pv[:])
                                with (
                                    tc.tile_pool(name="ps_st2", bufs=2, space="PSUM") as ps_st2,
                                    tc.tile_pool(name="ps_pv", bufs=2, space="PSUM") as ps_pv,
                                ):
                                    for b in range(4):
                                        bs = slice(b * 512, (b + 1) * 512)
                                        at_ps = ps_pv.tile([P, 512], f32, tag="at")
                                        rsum = ps_pv.tile([1, 512], f32, tag="rsum")
                                        n_i_tiles = 4 * b + 4
                                        for i in range(n_i_tiles):
                                            moff = max(0, (i - 4 * b) * P)
                                            nv = 512 - moff
                                            st_ps = ps_st2.tile([P, 512], f32, tag="st2")
                                            nc.tensor.matmul(
                                                st_ps[:, 0:nv],
                                                xk[:, i * P : (i + 1) * P],
                                                xq[:, b * 512 + moff : (b + 1) * 512],
                                                start=True, stop=True)
                                            E = p_E.tile([P, 512], f32r, tag="E")
                                            if moff:
                                                nc.vector.tensor_copy(
                                                    E[:, 0:moff], zeros_r[:, 0:moff])
                                            nc.scalar.activation(
                                                E[:, moff:512], st_ps[:, 0:nv], AF.Exp,
                                                scale=SCALE)
                                            if i >= 4 * b:
                                                nc.gpsimd.tensor_mul(
                                                    E[:, moff : moff + P],
                                                    E[:, moff : moff + P],
                                                    triu[:])
                                            nc.tensor.matmul(
                                                at_ps[:], v_sb[:, i, :], E[:],
                                                start=(i == 0), stop=(i == n_i_tiles - 1))
                                            nc.tensor.matmul(
                                                rsum[:], ones_c[:], E[:],
                                                start=(i == 0), stop=(i == n_i_tiles - 1))
                                        recip = p_rec.tile([1, 512], f32, tag="recip")
                                        nc.vector.reciprocal(recip[:], rsum[:])
                                        recb = p_rec.tile([P, 512], f32, tag="recb")
                                        nc.gpsimd.partition_broadcast(recb[:], recip[:])
                                        nc.vector.tensor_mul(attnT[:, h, bs], at_ps[:], recb[:])

                    if dump:
                        nc.sync.dma_start(d_attnT[:], attnT[:].bitcast(f32))

                    if "3" in phases:
                        if "2" not in phases:
                            nc.sync.dma_start(attnT[:], attnT_in[:].bitcast(f32r))
                        # ---------------- P3: o_proj ----------------
                        with (
                            tc.tile_pool(name="p3_wld", bufs=2) as p3_wld,
                            tc.tile_pool(name="p3_wT", bufs=2) as p3_wT,
                            tc.tile_pool(name="p3_o", bufs=3) as p3_o,
                            tc.tile_pool(name="ps_o", bufs=2, space="PSUM") as ps_o,
                        ):
                            for nb in range(HID // 512):
                                wold = p3_wld.tile([P, 4, 512], f32r, tag="wold")
                                nc.sync.dma_start(
                                    wold[:],
                                    wo[nb * 512 : (nb + 1) * 512, :]
                                    .rearrange("(a p) c -> p a c", p=P)
                                    .bitcast(f32r),
                                )
                                woT = p3_wT.tile([P, 4, 512], f32r, tag="woT")
                                for c in range(4):
                                    pw = ps_misc.tile([P, 4, P], f32r, tag="misc")
                                    for j in range(4):
                                        nc.tensor.transpose(
                                            pw[:, j, :],
                                            wold[:, j, c * P : (c + 1) * P],
                                            ident[:],
                                        )
                                    nc.any.tensor_copy(
                                        woT[:, c, :],
                                        pw[:].rearrange("p a b -> p (a b)"),
                                    )
                                for tg in range(T // (4 * P)):
                                    o_sb = p3_o.tile([P, 4, 512], f32, tag="o_sb")
                                    for j in range(4):
                                        t = tg * 4 + j
                                        po = ps_o.tile([P, 512], f32, tag="po")
                                        for c in range(4):
                                            nc.tensor.matmul(
                                                po[:],
                                                attnT[:, c, t * P : (t + 1) * P],
                                                woT[:, c, :],
                                                start=(c == 0), stop=(c == 3))
                                        nc.any.tensor_copy(o_sb[:, j, :], po[:])
                                    nc.sync.dma_start(
                                        out[tg * 4 * P : (tg + 1) * 4 * P,
                                            nb * 512 : (nb + 1) * 512]
                                        .rearrange("(a p) n -> p a n", p=P),
                                        o_sb[:])


            if n_iters == 1:
                _phases()
            else:
                with tc.For_i(0, n_iters, 1) as _iv:
                    _phases(_iv)

    nc.compile()
    return nc


def _get_nc(n_iters: int = 1):
    if n_iters not in _NC_CACHE:
        _NC_CACHE[n_iters] = build_nc(n_iters)
    return _NC_CACHE[n_iters]


def _shard_inputs(positions, hidden_states, w_qkv, w_o, q_norm_w, q_norm_b,
                  k_norm_w, k_norm_b):
    H = 32
    in_maps = []
    for c in range(8):
        hs = slice(c * H_PER_CORE, (c + 1) * H_PER_CORE)
        rows = np.concatenate(
            [
                w_qkv[c * 512 : (c + 1) * 512],
                w_qkv[H * D + c * 512 : H * D + (c + 1) * 512],
                w_qkv[2 * H * D + c * 512 : 2 * H * D + (c + 1) * 512],
            ],
            axis=0,
        )
        in_maps.append(
            {
                "hidden": np.ascontiguousarray(hidden_states, dtype=np.float32),
                "wq": np.ascontiguousarray(rows, dtype=np.float32),
                "wo": np.ascontiguousarray(w_o[:, c * 512 : (c + 1) * 512],
                                           dtype=np.float32),
                "pos": np.ascontiguousarray(positions, dtype=np.int32),
                "qnw": np.ascontiguousarray(q_norm_w[hs], dtype=np.float32),
                "qnb": np.ascontiguousarray(q_norm_b[hs], dtype=np.float32),
                "knw": np.ascontiguousarray(k_norm_w[hs], dtype=np.float32),
                "knb": np.ascontiguousarray(k_norm_b[hs], dtype=np.float32),
            }
        )
    return in_maps


def kernel(positions, hidden_states, w_qkv, w_o, q_norm_w, q_norm_b,
           k_norm_w, k_norm_b):
    nc = _get_nc(1)
    in_maps = _shard_inputs(
        np.asarray(positions), np.asarray(hidden_states), np.asarray(w_qkv),
        np.asarray(w_o), np.asarray(q_norm_w), np.asarray(q_norm_b),
        np.asarray(k_norm_w), np.asarray(k_norm_b),
    )
    res = run_bass_kernel_spmd(nc, in_maps, list(range(8))).results
    acc = np.zeros((T, HID), np.float64)
    for c in range(8):
        acc += res[c]["out"].astype(np.float64)
    return acc.astype(np.float32)


if __name__ == "__main__":
    build_nc(1)
    print("build OK")



# revision 6
# speedup vs baseline: 1.5592x; 1.5592x over previous
"""ChameleonAttention Trainium2 kernel (v2).

Full-input contract: kernel(**inputs) with the complete tensors; internally
shards tensor-parallel across 8 NeuronCores by attention head (4 heads/core):
  - w_qkv rows + q/k norm params sharded by head
  - w_o columns sharded by head, partial outputs summed on host (all-reduce)

v2 design (vs v1): qkv is produced in NATURAL [t, cols] layout so that
LayerNorm stats live on partitions (no single-partition [1,N] DVE ops) and
RoPE is pure free-dim arithmetic (no cross-partition copies). All PE operands
are bf16 (1 cyc/row, incl. N<256), everything stays SBUF-resident (no DRAM
scratch round-trip), K is split in halves with the f32 partial kept in SBUF.
Activation tables are sequenced Exp(invf) -> Sin -> Sqrt -> Exp(attn) with no
interleaving, and softmax denominators use reciprocal_approx_fast.

Per-core dataflow:
  P1: for kh in {0,1}: transpose w half (PE, f32r); per 256-token group:
      transpose hidden half, qkv_nat[t, 1536] += hT.T @ wT via PSUM
      accumulation over 16 k-tiles (N=512 matmuls, stationary=hT tile).
      After kh=1: LN (ACT Square+accum / DVE reduce, per-partition stats),
      affine (broadcast w,b tiles), neox RoPE (free-dim halves, device
      sin/cos with Cody-Waite), PE-transpose q,k into qkT[d, t]; v copied
      natural into v_nat[t, d].
  P2: causal attention per (head, q-block): S^T tiles on PE, exp on ACT
      (bf16 E), triu mask on DVE, P@V + ones-rsum accumulated in PSUM,
      denominator via reciprocal_approx_fast + partition_broadcast.
  P3: o_proj: out[t, :] = attnT.T @ woT with PE-transposed w_o tiles (bf16).
"""
import sys

sys.path.insert(0, "/opt/trn_rl_repo")

import numpy as np

import concourse.bass as bass
import concourse.mybir as mybir
import concourse.tile as tile
from concourse import bacc
from concourse.bass_utils import run_bass_kernel_spmd
from concourse.masks import make_identity, make_upper_triangular

P = 128
T = 2048
HID = 4096
D = 128
HPC = 4  # heads per core
R = 3 * HPC  # 12 qkv row-tiles per core
KH = HID // 2  # 2048, contraction half
NKK = KH // P  # 16 k-tiles per half
TG = 256  # token group
NTG = T // TG  # 8
THETA = 10000.0
EPS = 1e-5
SCALE = D ** -0.5
TWO_PI = 6.283185307179586
C_HI = float(np.float32(6.28125))
C_LO = TWO_PI - C_HI

f32 = mybir.dt.float32
f32r = mybir.dt.float32r
bf16 = mybir.dt.bfloat16
i32 = mybir.dt.int32
AF = mybir.ActivationFunctionType
ALU = mybir.AluOpType

_NC_CACHE = {}


def build_nc(dump: bool = False):
    nc = bacc.Bacc(None, target_bir_lowering=False, debug=False)

    hidden = nc.dram_tensor("hidden", (T, HID), f32, kind="ExternalInput")
    wq = nc.dram_tensor("wq", (R * P, HID), f32, kind="ExternalInput")
    wo = nc.dram_tensor("wo", (HID, HPC * D), f32, kind="ExternalInput")
    pos = nc.dram_tensor("pos", (T,), i32, kind="ExternalInput")
    qnw = nc.dram_tensor("qnw", (HPC, D), f32, kind="ExternalInput")
    qnb = nc.dram_tensor("qnb", (HPC, D), f32, kind="ExternalInput")
    knw = nc.dram_tensor("knw", (HPC, D), f32, kind="ExternalInput")
    knb = nc.dram_tensor("knb", (HPC, D), f32, kind="ExternalInput")
    out = nc.dram_tensor("out", (T, HID), f32, kind="ExternalOutput")
    if dump:
        d_qkT = nc.dram_tensor("d_qkT", (P, 8, T), bf16, kind="ExternalOutput")
        d_vnat = nc.dram_tensor("d_vnat", (P, T // P, 512), bf16, kind="ExternalOutput")
        d_attnT = nc.dram_tensor("d_attnT", (P, HPC, T), bf16, kind="ExternalOutput")

    with tile.TileContext(nc) as tc:
        with tc.tile_pool(name="const", bufs=1) as const:
            # --- constants ---
            ident_r = const.tile([P, P], f32r)
            ident_b = const.tile([P, P], bf16)
            triu_b = const.tile([P, P], bf16)
            ones_c = const.tile([P, 1], bf16)
            epsc = const.tile([P, 1], f32)
            nc.vector.memset(epsc[:], EPS)
            w8 = const.tile([P, 8, D], bf16)  # LN weight, bcast over t
            b8 = const.tile([P, 8, D], bf16)
            c4 = const.tile([P, T // P, HPC, 64], bf16)  # cos, replicated x4 heads
            s4 = const.tile([P, T // P, HPC, 64], bf16)
            qkT = const.tile([P, 8, T], bf16)  # post-rope q(0:4), k(4:8); [d, t]
            v_nat = const.tile([P, T // P, 512], bf16)  # [t, 4 heads * d]
            qkv_bf = const.tile([P, T // P, R * P], bf16)  # natural qkv accum

            with tc.tile_pool(name="cstage", bufs=1) as cstage:
                ident_f = cstage.tile([P, P], f32)
                make_identity(nc, ident_f[:])
                nc.vector.tensor_copy(ident_r[:], ident_f[:])
                nc.vector.tensor_copy(ident_b[:], ident_f[:])
                triu_f = cstage.tile([P, P], f32)
                make_upper_triangular(nc, triu_f[:], val=1.0, diag=True)
                nc.vector.tensor_copy(triu_b[:], triu_f[:])
                ones_f = cstage.tile([P, 1], f32)
                nc.vector.memset(ones_f[:], 1.0)
                nc.vector.tensor_copy(ones_c[:], ones_f[:])

                # LN affine params broadcast over partitions: w8/b8[p, g*4+h, :]
                nrm = cstage.tile([1, 8, D], f32)
                for h in range(HPC):
                    nc.sync.dma_start(nrm[:, h, :], qnw[h : h + 1, :])
                    nc.sync.dma_start(nrm[:, 4 + h, :], knw[h : h + 1, :])
                nrm_b = cstage.tile([P, 8, D], f32)
                nc.gpsimd.partition_broadcast(nrm_b[:], nrm[:])
                nc.vector.tensor_copy(w8[:], nrm_b[:])
                nrm2 = cstage.tile([1, 8, D], f32, tag="nrm2")
                for h in range(HPC):
                    nc.sync.dma_start(nrm2[:, h, :], qnb[h : h + 1, :])
                    nc.sync.dma_start(nrm2[:, 4 + h, :], knb[h : h + 1, :])
                nrm2_b = cstage.tile([P, 8, D], f32, tag="nrm2b")
                nc.gpsimd.partition_broadcast(nrm2_b[:], nrm2[:])
                nc.vector.tensor_copy(b8[:], nrm2_b[:])

            # --- rope tables in natural layout: [t(part), tt, 64] ---
            with tc.tile_pool(name="rtmp", bufs=1) as rtmp:
                NTT = T // P  # 16
                jj = rtmp.tile([1, 64], f32)
                nc.gpsimd.iota(jj[:], pattern=[[1, 64]], base=0,
                               channel_multiplier=0,
                               allow_small_or_imprecise_dtypes=True)
                invf = rtmp.tile([1, 64], f32)
                nc.scalar.activation(invf[:], jj[:], AF.Exp,
                                     scale=-float(np.log(THETA)) / 64.0)
                invf_b = rtmp.tile([P, 64], f32)
                nc.gpsimd.partition_broadcast(invf_b[:], invf[:])
                # t values from positions input: tval[p, i] = pos[i*128 + p]
                pos_i = rtmp.tile([P, NTT], i32)
                nc.sync.dma_start(pos_i[:], pos.rearrange("(i p) -> p i", p=P))
                tval = rtmp.tile([P, NTT], f32)
                nc.vector.tensor_copy(tval[:], pos_i[:])
                freqs = rtmp.tile([P, NTT, 64], f32)
                for i in range(NTT):
                    nc.vector.tensor_scalar_mul(freqs[:, i, :], invf_b[:],
                                                tval[:, i : i + 1])

                def reduced_sin(dst_ap, src_ap):
                    # dst = sin(reduce(src)), reduce(x) = x - 2pi*round(x/2pi)
                    q = rtmp.tile([P, NTT, 64], f32, tag="rs_q")
                    nc.vector.tensor_scalar_mul(q[:], src_ap, 1.0 / TWO_PI)
                    n_i = rtmp.tile([P, NTT, 64], i32, tag="rs_n")
                    nc.vector.tensor_copy(n_i[:], q[:])  # round-to-nearest
                    n_f = rtmp.tile([P, NTT, 64], f32, tag="rs_nf")
                    nc.vector.tensor_copy(n_f[:], n_i[:])
                    r0 = rtmp.tile([P, NTT, 64], f32, tag="rs_r0")
                    nc.vector.scalar_tensor_tensor(
                        out=r0[:], in0=n_f[:], scalar=-C_HI, in1=src_ap,
                        op0=ALU.mult, op1=ALU.add)
                    r1 = rtmp.tile([P, NTT, 64], f32, tag="rs_r1")
                    nc.vector.scalar_tensor_tensor(
                        out=r1[:], in0=n_f[:], scalar=-C_LO, in1=r0[:],
                        op0=ALU.mult, op1=ALU.add)
                    nc.scalar.activation(dst_ap, r1[:], AF.Sin)

                sc1 = rtmp.tile([P, NTT, 64], bf16, tag="sc1")
                reduced_sin(sc1[:], freqs[:])
                for h in range(HPC):
                    nc.vector.tensor_copy(s4[:, :, h, :], sc1[:])
                fr2 = rtmp.tile([P, NTT, 64], f32, tag="fr2")
                nc.vector.tensor_scalar_add(fr2[:], freqs[:], np.pi / 2)
                sc2 = rtmp.tile([P, NTT, 64], bf16, tag="sc2")
                reduced_sin(sc2[:], fr2[:])
                for h in range(HPC):
                    nc.vector.tensor_copy(c4[:, :, h, :], sc2[:])

            # ---------------- P1 + LN + RoPE ----------------
            with (
                tc.tile_pool(name="p1_stage", bufs=2) as p_stage,
                tc.tile_pool(name="p1_wT", bufs=1) as p_wT,
                tc.tile_pool(name="p1_hT", bufs=2) as p_hT,
                tc.tile_pool(name="p1_scr", bufs=2) as p_scr,
                tc.tile_pool(name="p1_st", bufs=2) as p_st,
                tc.tile_pool(name="ps_tp", bufs=2, space="PSUM") as ps_tp,
                tc.tile_pool(name="ps_acc", bufs=1, space="PSUM") as ps_acc,
            ):
                for kh in range(2):
                    k0 = kh * KH
                    # transpose w half -> wT[k, kk, 1536 cols]
                    wT = p_wT.tile([P, NKK, R * P], bf16, tag="wT")
                    for rt in range(R):
                        wst = p_stage.tile([P, KH], f32r, tag="stage")
                        nc.sync.dma_start(
                            wst[:], wq[rt * P : (rt + 1) * P, k0 : k0 + KH].bitcast(f32r)
                        )
                        for kg in range(NKK // 4):
                            pw = ps_tp.tile([P, 4, P], f32r, tag="tp")
                            for j in range(4):
                                kk = kg * 4 + j
                                nc.tensor.transpose(
                                    pw[:, j, :], wst[:, kk * P : (kk + 1) * P],
                                    ident_r[:])
                            nc.any.tensor_copy(
                                wT[:, kg * 4 : kg * 4 + 4, rt * P : (rt + 1) * P],
                                pw[:])
                    for tg in range(NTG):
                        t0 = tg * TG
                        # transpose hidden half for this token group
                        hT = p_hT.tile([P, NKK, TG], bf16, tag="hT")
                        for tt in range(TG // P):
                            hst = p_stage.tile([P, KH], f32r, tag="stage")
                            nc.sync.dma_start(
                                hst[:],
                                hidden[t0 + tt * P : t0 + (tt + 1) * P,
                                       k0 : k0 + KH].bitcast(f32r))
                            for kg in range(NKK // 4):
                                ph = ps_tp.tile([P, 4, P], f32r, tag="tp")
                                for j in range(4):
                                    kk = kg * 4 + j
                                    nc.tensor.transpose(
                                        ph[:, j, :], hst[:, kk * P : (kk + 1) * P],
                                        ident_r[:])
                                nc.any.tensor_copy(
                                    hT[:, kg * 4 : kg * 4 + 4, tt * P : (tt + 1) * P],
                                    ph[:])
                        # qkv_nat[t, 1536] partial for this group
                        for tt in range(TG // P):
                            ti_e = tg * (TG // P) + tt
                            for cb in range(3):
                                acc = ps_acc.tile([P, 512], f32, tag=f"acc{cb}")
                                for kk in range(NKK):
                                    nc.tensor.matmul(
                                        acc[:],
                                        hT[:, kk, tt * P : (tt + 1) * P],
                                        wT[:, kk, cb * 512 : (cb + 1) * 512],
                                        start=(kk == 0), stop=(kk == NKK - 1))
                                dst = qkv_bf[:, ti_e, cb * 512 : (cb + 1) * 512]
                                if kh == 0:
                                    nc.any.tensor_copy(dst, acc[:])
                                else:
                                    nc.vector.tensor_add(dst, dst, acc[:])
                        if kh == 0:
                            continue
                        # ---- LN + affine + rope + transpose (full qkv now) ----
                        for tt in range(TG // P):
                            ta = t0 + tt * P  # absolute t-tile start
                            ti = ta // P
                            nc.any.tensor_copy(v_nat[:, ti, :],
                                               qkv_bf[:, ti, 2 * 512 : 3 * 512])
                            for grp in range(2):  # 0: q heads, 1: k heads
                                xg = qkv_bf[:, ti, grp * 512 : (grp + 1) * 512]
                                xg4 = xg.rearrange("p (h d) -> p h d", h=HPC)
                                s1 = p_st.tile([P, HPC], f32, tag="s1")
                                s2 = p_st.tile([P, HPC], f32, tag="s2")
                                sqs = p_scr.tile([P, 512], bf16, tag="sqs")
                                for h in range(HPC):
                                    nc.scalar.activation(
                                        sqs[:, h * D : (h + 1) * D],
                                        xg4[:, h, :], AF.Square,
                                        accum_out=s2[:, h : h + 1])
                                nc.vector.tensor_reduce(
                                    s1[:], xg4, axis=mybir.AxisListType.X,
                                    op=ALU.add)
                                mu = p_st.tile([P, HPC], f32, tag="mu")
                                nc.vector.tensor_scalar_mul(mu[:], s1[:], 1.0 / D)
                                musq = p_st.tile([P, HPC], f32, tag="musq")
                                nc.vector.tensor_mul(musq[:], mu[:], mu[:])
                                varv = p_st.tile([P, HPC], f32, tag="varv")
                                nc.vector.scalar_tensor_tensor(
                                    out=varv[:], in0=s2[:], scalar=1.0 / D,
                                    in1=musq[:], op0=ALU.mult, op1=ALU.subtract)
                                stdv = p_st.tile([P, HPC], f32, tag="stdv")
                                nc.scalar.activation(stdv[:], varv[:], AF.Sqrt,
                                                     bias=epsc[:])
                                rstd = p_st.tile([P, HPC], f32, tag="rstd")
                                nc.vector.reciprocal(rstd[:], stdv[:])
                                y = p_scr.tile([P, 512], bf16, tag="y")
                                for h in range(HPC):
                                    nc.vector.tensor_scalar(
                                        out=y[:, h * D : (h + 1) * D],
                                        in0=xg4[:, h, :],
                                        scalar1=mu[:, h : h + 1],
                                        scalar2=rstd[:, h : h + 1],
                                        op0=ALU.subtract, op1=ALU.mult)
                                y4 = y.rearrange("p (h d) -> p h d", h=HPC)
                                nc.vector.tensor_mul(
                                    y4, y4, w8[:, grp * 4 : grp * 4 + 4, :])
                                nc.vector.tensor_add(
                                    y4, y4, b8[:, grp * 4 : grp * 4 + 4, :])
                                # rope: halves along d
                                yh = y.rearrange("p (h v d) -> p h v d", h=HPC, v=2)
                                ro = p_scr.tile([P, 512], bf16, tag="ro")
                                roh = ro.rearrange("p (h v d) -> p h v d", h=HPC, v=2)
                                tmp = p_scr.tile([P, 512], bf16, tag="tmp")
                                tmph = tmp.rearrange("p (h v d) -> p h v d", h=HPC, v=2)
                                cc = c4[:, ti, :, :]
                                ss = s4[:, ti, :, :]
                                nc.vector.tensor_mul(tmph[:, :, 0, :], yh[:, :, 0, :], cc)
                                nc.vector.tensor_mul(tmph[:, :, 1, :], yh[:, :, 1, :], ss)
                                nc.vector.tensor_sub(roh[:, :, 0, :], tmph[:, :, 0, :],
                                                     tmph[:, :, 1, :])
                                nc.vector.tensor_mul(tmph[:, :, 0, :], yh[:, :, 1, :], cc)
                                nc.vector.tensor_mul(tmph[:, :, 1, :], yh[:, :, 0, :], ss)
                                nc.vector.tensor_add(roh[:, :, 1, :], tmph[:, :, 0, :],
                                                     tmph[:, :, 1, :])
                                # transpose 4 heads -> qkT[d, grp*4+h, t]
                                pq = ps_tp.tile([P, 4, P], bf16, tag="tpb")
                                for h in range(HPC):
                                    nc.tensor.transpose(
                                        pq[:, h, :], ro[:, h * D : (h + 1) * D],
                                        ident_b[:])
                                nc.any.tensor_copy(
                                    qkT[:, grp * 4 : grp * 4 + 4, ta : ta + P],
                                    pq[:])

            if dump:
                nc.sync.dma_start(d_qkT[:], qkT[:])
                nc.sync.dma_start(d_vnat[:], v_nat[:])

            # ---------------- P2: causal attention ----------------
            with tc.tile_pool(name="p2_attnT", bufs=1) as p_attnT:
                attnT = p_attnT.tile([P, HPC, T], bf16, tag="attnT")
                with (
                    tc.tile_pool(name="p2_E", bufs=3) as p_E,
                    tc.tile_pool(name="p2_tail", bufs=2) as p_tail,
                    tc.tile_pool(name="ps_st", bufs=2, space="PSUM") as ps_st,
                    tc.tile_pool(name="ps_at", bufs=2, space="PSUM") as ps_at,
                ):
                    for h in range(HPC):
                        for b in range(4):
                            bs = slice(b * 512, (b + 1) * 512)
                            at_ps = ps_at.tile([P, 512], f32, tag="at")
                            rsum = ps_at.tile([1, 512], f32, tag="rsum")
                            n_i = 4 * b + 4
                            for i in range(n_i):
                                moff = max(0, (i - 4 * b) * P)
                                nv = 512 - moff
                                st = ps_st.tile([P, 512], f32, tag="st")
                                nc.tensor.matmul(
                                    st[:, 0:nv],
                                    qkT[:, 4 + h, i * P : (i + 1) * P],
                                    qkT[:, h, b * 512 + moff : (b + 1) * 512],
                                    start=True, stop=True)
                                E = p_E.tile([P, 512], bf16, tag="E")
                                if moff:
                                    nc.vector.memset(E[:, 0:moff], 0.0)
                                nc.scalar.activation(E[:, moff:512], st[:, 0:nv],
                                                     AF.Exp, scale=SCALE)
                                if i >= 4 * b:
                                    nc.vector.tensor_mul(
                                        E[:, moff : moff + P],
                                        E[:, moff : moff + P], triu_b[:])
                                nc.tensor.matmul(
                                    at_ps[:], v_nat[:, i, h * D : (h + 1) * D],
                                    E[:], start=(i == 0), stop=(i == n_i - 1))
                                nc.tensor.matmul(
                                    rsum[:], ones_c[:], E[:],
                                    start=(i == 0), stop=(i == n_i - 1))
                            rs_sb = p_tail.tile([1, 512], f32, tag="rs_sb")
                            nc.scalar.activation(rs_sb[:], rsum[:], AF.Copy)
                            rc = p_tail.tile([1, 512], f32, tag="rc")
                            nc.vector.reciprocal_approx_fast(out=rc[:], in_=rs_sb[:])
                            recb = p_tail.tile([P, 512], f32, tag="recb")
                            nc.gpsimd.partition_broadcast(recb[:], rc[:])
                            nc.vector.tensor_mul(attnT[:, h, bs], at_ps[:], recb[:])

                if dump:
                    nc.sync.dma_start(d_attnT[:], attnT[:])

                # ---------------- P3: o_proj ----------------
                with (
                    tc.tile_pool(name="p3_wld", bufs=2) as p3_wld,
                    tc.tile_pool(name="p3_wT", bufs=2) as p3_wT,
                    tc.tile_pool(name="p3_o", bufs=3) as p3_o,
                    tc.tile_pool(name="ps_o", bufs=2, space="PSUM") as ps_o,
                    tc.tile_pool(name="ps_tp3", bufs=2, space="PSUM") as ps_tp3,
                ):
                    for nb in range(HID // 512):
                        wold = p3_wld.tile([P, 4, 512], f32r, tag="wold")
                        nc.sync.dma_start(
                            wold[:],
                            wo[nb * 512 : (nb + 1) * 512, :]
                            .rearrange("(a p) c -> p a c", p=P)
                            .bitcast(f32r))
                        woT = p3_wT.tile([P, 4, 512], bf16, tag="woT")
                        for c in range(4):
                            pw = ps_tp3.tile([P, 4, P], f32r, tag="tp3")
                            for j in range(4):
                                nc.tensor.transpose(
                                    pw[:, j, :], wold[:, j, c * P : (c + 1) * P],
                                    ident_r[:])
                            nc.any.tensor_copy(
                                woT[:, c, :],
                                pw[:].rearrange("p a b -> p (a b)"))
                        for tg4 in range(T // 512):
                            o_sb = p3_o.tile([P, 4, 512], f32, tag="o_sb")
                            for j in range(4):
                                t = tg4 * 4 + j
                                po = ps_o.tile([P, 512], f32, tag="po")
                                for c in range(4):
                                    nc.tensor.matmul(
                                        po[:],
                                        attnT[:, c, t * P : (t + 1) * P],
                                        woT[:, c, :],
                                        start=(c == 0), stop=(c == 3))
                                nc.any.tensor_copy(o_sb[:, j, :], po[:])
                            nc.sync.dma_start(
                                out[tg4 * 512 : (tg4 + 1) * 512,
                                    nb * 512 : (nb + 1) * 512]
                                .rearrange("(a p) n -> p a n", p=P),
                                o_sb[:])

    nc.compile()
    return nc


def _get_nc():
    if "nc" not in _NC_CACHE:
        _NC_CACHE["nc"] = build_nc()
    return _NC_CACHE["nc"]


def _shard_inputs(positions, hidden_states, w_qkv, w_o, q_norm_w, q_norm_b,
                  k_norm_w, k_norm_b):
    H = 32
    in_maps = []
    for c in range(8):
        hs = slice(c * HPC, (c + 1) * HPC)
        rows = np.concatenate(
            [
                w_qkv[c * 512 : (c + 1) * 512],
                w_qkv[H * D + c * 512 : H * D + (c + 1) * 512],
                w_qkv[2 * H * D + c * 512 : 2 * H * D + (c + 1) * 512],
            ],
            axis=0,
        )
        in_maps.append(
            {
                "hidden": np.ascontiguousarray(hidden_states, dtype=np.float32),
                "wq": np.ascontiguousarray(rows, dtype=np.float32),
                "wo": np.ascontiguousarray(w_o[:, c * 512 : (c + 1) * 512],
                                           dtype=np.float32),
                "pos": np.ascontiguousarray(positions, dtype=np.int32),
                "qnw": np.ascontiguousarray(q_norm_w[hs], dtype=np.float32),
                "qnb": np.ascontiguousarray(q_norm_b[hs], dtype=np.float32),
                "knw": np.ascontiguousarray(k_norm_w[hs], dtype=np.float32),
                "knb": np.ascontiguousarray(k_norm_b[hs], dtype=np.float32),
            }
        )
    return in_maps


def kernel(positions, hidden_states, w_qkv, w_o, q_norm_w, q_norm_b,
           k_norm_w, k_norm_b):
    nc = _get_nc()
    in_maps = _shard_inputs(
        np.asarray(positions), np.asarray(hidden_states), np.asarray(w_qkv),
        np.asarray(w_o), np.asarray(q_norm_w), np.asarray(q_norm_b),
        np.asarray(k_norm_w), np.asarray(k_norm_b),
    )
    res = run_bass_kernel_spmd(nc, in_maps, list(range(8))).results
    acc = np.zeros((T, HID), np.float64)
    for c in range(8):
        acc += res[c]["out"].astype(np.float64)
    return acc.astype(np.float32)


if __name__ == "__main__":
    build_nc()
    print("build OK")


# revision 7
# speedup vs baseline: 1.9634x; 1.2592x over previous
"""ChameleonAttention Trainium2 kernel (v3).

Full-input contract: kernel(**inputs) with the complete tensors; internally
shards tensor-parallel across 8 NeuronCores by attention head (4 heads/core):
  - w_qkv rows + q/k norm params sharded by head
  - w_o columns sharded by head, partial outputs summed on host (all-reduce)

v3: host pre-transposes and pre-converts the big operands to bf16
(hidden^T, w_qkv^T, w_o^T), so the device does ZERO weight/activation
transposes in P1/P3 — tiles DMA straight into matmul-ready layouts. qkv is
produced in natural [t, cols] layout (per-partition LN stats, free-dim RoPE),
attention uses S^T tiles with bf16 E, and output partials are written bf16
(summed f64 on host).

Per-core dataflow:
  P1: for kh in {0,1}: DMA wT half [k, 16kk, 1536]; per 256-token group:
      DMA hT [k, 16kk, 256], qkv_nat[t,1536] += hT.T @ wT (PSUM over 16 kk,
      N=512, stationary=hT tile). After kh=1: LN (ACT Square+accum / DVE
      reduce), affine, neox RoPE (free-dim halves, Cody-Waite sin/cos),
      PE-transpose q,k into qkT[d,t]; v copied natural.
  P2: causal attention per (head, q-block): S^T on PE, exp on ACT (bf16 E),
      triu mask on DVE, P@V + ones-rsum in PSUM, reciprocal_approx_fast +
      partition_broadcast for the denominator.
  P3: out[t,:] = attnT.T @ woT, woT DMA'd directly; bf16 partial out.
"""
import sys

sys.path.insert(0, "/opt/trn_rl_repo")

import numpy as np
import ml_dtypes

import concourse.bass as bass
import concourse.mybir as mybir
import concourse.tile as tile
from concourse import bacc
from concourse.bass_utils import run_bass_kernel_spmd
from concourse.masks import make_identity, make_upper_triangular

P = 128
T = 2048
HID = 4096
D = 128
HPC = 4  # heads per core
R = 3 * HPC  # 12 qkv row-tiles per core
KH = HID // 2  # 2048, contraction half
NKK = KH // P  # 16 k-tiles per half
TG = 256  # token group
NTG = T // TG  # 8
THETA = 10000.0
EPS = 1e-5
SCALE = D ** -0.5
TWO_PI = 6.283185307179586
C_HI = float(np.float32(6.28125))
C_LO = TWO_PI - C_HI

f32 = mybir.dt.float32
bf16 = mybir.dt.bfloat16
i32 = mybir.dt.int32
AF = mybir.ActivationFunctionType
ALU = mybir.AluOpType

_NC_CACHE = {}


def build_nc(dump: bool = False):
    nc = bacc.Bacc(None, target_bir_lowering=False, debug=False)

    hiddenT = nc.dram_tensor("hiddenT", (HID, T), bf16, kind="ExternalInput")
    wqT = nc.dram_tensor("wqT", (HID, R * P), bf16, kind="ExternalInput")
    woT = nc.dram_tensor("woT", (HPC * D, HID), bf16, kind="ExternalInput")
    pos = nc.dram_tensor("pos", (T,), i32, kind="ExternalInput")
    qnw = nc.dram_tensor("qnw", (HPC, D), f32, kind="ExternalInput")
    qnb = nc.dram_tensor("qnb", (HPC, D), f32, kind="ExternalInput")
    knw = nc.dram_tensor("knw", (HPC, D), f32, kind="ExternalInput")
    knb = nc.dram_tensor("knb", (HPC, D), f32, kind="ExternalInput")
    out = nc.dram_tensor("out", (T, HID), bf16, kind="ExternalOutput")
    if dump:
        d_qkT = nc.dram_tensor("d_qkT", (P, 8, T), bf16, kind="ExternalOutput")
        d_vnat = nc.dram_tensor("d_vnat", (P, T // P, 512), bf16, kind="ExternalOutput")
        d_attnT = nc.dram_tensor("d_attnT", (P, HPC, T), bf16, kind="ExternalOutput")

    with tile.TileContext(nc) as tc:
        with tc.tile_pool(name="const", bufs=1) as const:
            # --- constants ---
            ident_b = const.tile([P, P], bf16)
            triu_b = const.tile([P, P], bf16)
            ones_c = const.tile([P, 1], bf16)
            epsc = const.tile([P, 1], f32)
            nc.vector.memset(epsc[:], EPS)
            w8 = const.tile([P, 8, D], bf16)  # LN weight, bcast over t
            b8 = const.tile([P, 8, D], bf16)
            c4 = const.tile([P, T // P, HPC, 64], bf16)  # cos, replicated x4 heads
            s4 = const.tile([P, T // P, HPC, 64], bf16)
            qkT = const.tile([P, 8, T], bf16)  # post-rope q(0:4), k(4:8); [d, t]
            v_nat = const.tile([P, T // P, 512], bf16)  # [t, 4 heads * d]
            qkv_bf = const.tile([P, T // P, R * P], bf16)  # natural qkv accum

            with tc.tile_pool(name="cstage", bufs=1) as cstage:
                ident_f = cstage.tile([P, P], f32)
                make_identity(nc, ident_f[:])
                nc.vector.tensor_copy(ident_b[:], ident_f[:])
                triu_f = cstage.tile([P, P], f32)
                make_upper_triangular(nc, triu_f[:], val=1.0, diag=True)
                nc.vector.tensor_copy(triu_b[:], triu_f[:])
                ones_f = cstage.tile([P, 1], f32)
                nc.vector.memset(ones_f[:], 1.0)
                nc.vector.tensor_copy(ones_c[:], ones_f[:])

                # LN affine params broadcast over partitions: w8/b8[p, g*4+h, :]
                nrm = cstage.tile([1, 8, D], f32)
                for h in range(HPC):
                    nc.sync.dma_start(nrm[:, h, :], qnw[h : h + 1, :])
                    nc.sync.dma_start(nrm[:, 4 + h, :], knw[h : h + 1, :])
                nrm_b = cstage.tile([P, 8, D], f32)
                nc.gpsimd.partition_broadcast(nrm_b[:], nrm[:])
                nc.vector.tensor_copy(w8[:], nrm_b[:])
                nrm2 = cstage.tile([1, 8, D], f32, tag="nrm2")
                for h in range(HPC):
                    nc.sync.dma_start(nrm2[:, h, :], qnb[h : h + 1, :])
                    nc.sync.dma_start(nrm2[:, 4 + h, :], knb[h : h + 1, :])
                nrm2_b = cstage.tile([P, 8, D], f32, tag="nrm2b")
                nc.gpsimd.partition_broadcast(nrm2_b[:], nrm2[:])
                nc.vector.tensor_copy(b8[:], nrm2_b[:])

            # --- rope tables in natural layout: [t(part), tt, 64] ---
            with tc.tile_pool(name="rtmp", bufs=1) as rtmp:
                NTT = T // P  # 16
                jj = rtmp.tile([1, 64], f32)
                nc.gpsimd.iota(jj[:], pattern=[[1, 64]], base=0,
                               channel_multiplier=0,
                               allow_small_or_imprecise_dtypes=True)
                invf = rtmp.tile([1, 64], f32)
                nc.scalar.activation(invf[:], jj[:], AF.Exp,
                                     scale=-float(np.log(THETA)) / 64.0)
                invf_b = rtmp.tile([P, 64], f32)
                nc.gpsimd.partition_broadcast(invf_b[:], invf[:])
                # t values from positions input: tval[p, i] = pos[i*128 + p]
                pos_i = rtmp.tile([P, NTT], i32)
                nc.sync.dma_start(pos_i[:], pos.rearrange("(i p) -> p i", p=P))
                tval = rtmp.tile([P, NTT], f32)
                nc.vector.tensor_copy(tval[:], pos_i[:])
                freqs = rtmp.tile([P, NTT, 64], f32)
                for i in range(NTT):
                    nc.vector.tensor_scalar_mul(freqs[:, i, :], invf_b[:],
                                                tval[:, i : i + 1])

                def reduced_sin(dst_ap, src_ap):
                    # dst = sin(reduce(src)), reduce(x) = x - 2pi*round(x/2pi)
                    q = rtmp.tile([P, NTT, 64], f32, tag="rs_q")
                    nc.vector.tensor_scalar_mul(q[:], src_ap, 1.0 / TWO_PI)
                    n_i = rtmp.tile([P, NTT, 64], i32, tag="rs_n")
                    nc.vector.tensor_copy(n_i[:], q[:])  # round-to-nearest
                    n_f = rtmp.tile([P, NTT, 64], f32, tag="rs_nf")
                    nc.vector.tensor_copy(n_f[:], n_i[:])
                    r0 = rtmp.tile([P, NTT, 64], f32, tag="rs_r0")
                    nc.vector.scalar_tensor_tensor(
                        out=r0[:], in0=n_f[:], scalar=-C_HI, in1=src_ap,
                        op0=ALU.mult, op1=ALU.add)
                    r1 = rtmp.tile([P, NTT, 64], f32, tag="rs_r1")
                    nc.vector.scalar_tensor_tensor(
                        out=r1[:], in0=n_f[:], scalar=-C_LO, in1=r0[:],
                        op0=ALU.mult, op1=ALU.add)
                    nc.scalar.activation(dst_ap, r1[:], AF.Sin)

                sc1 = rtmp.tile([P, NTT, 64], bf16, tag="sc1")
                reduced_sin(sc1[:], freqs[:])
                for h in range(HPC):
                    nc.vector.tensor_copy(s4[:, :, h, :], sc1[:])
                fr2 = rtmp.tile([P, NTT, 64], f32, tag="fr2")
                nc.vector.tensor_scalar_add(fr2[:], freqs[:], np.pi / 2)
                sc2 = rtmp.tile([P, NTT, 64], bf16, tag="sc2")
                reduced_sin(sc2[:], fr2[:])
                for h in range(HPC):
                    nc.vector.tensor_copy(c4[:, :, h, :], sc2[:])

            # ---------------- P1 + LN + RoPE ----------------
            with (
                tc.tile_pool(name="p1_wT", bufs=1) as p_wT,
                tc.tile_pool(name="p1_hT", bufs=2) as p_hT,
                tc.tile_pool(name="p1_scr", bufs=2) as p_scr,
                tc.tile_pool(name="p1_st", bufs=2) as p_st,
                tc.tile_pool(name="ps_tp", bufs=2, space="PSUM") as ps_tp,
                tc.tile_pool(name="ps_acc", bufs=2, space="PSUM") as ps_acc,
            ):
                for kh in range(2):
                    k0 = kh * KH
                    # wT half: [k(part), kk, 1536] direct DMA (4 chunks)
                    wT = p_wT.tile([P, NKK, R * P], bf16, tag="wT")
                    wsrc = wqT[k0 : k0 + KH, :].rearrange("(kk p) c -> p kk c", p=P)
                    for kg in range(4):
                        nc.sync.dma_start(wT[:, kg * 4 : (kg + 1) * 4, :],
                                          wsrc[:, kg * 4 : (kg + 1) * 4, :])
                    for tg in range(NTG):
                        t0 = tg * TG
                        hT = p_hT.tile([P, NKK, TG], bf16, tag="hT")
                        nc.sync.dma_start(
                            hT[:],
                            hiddenT[k0 : k0 + KH, t0 : t0 + TG]
                            .rearrange("(kk p) t -> p kk t", p=P))
                        for tt in range(TG // P):
                            ti_e = tg * (TG // P) + tt
                            for cb in range(3):
                                acc = ps_acc.tile([P, 512], f32, tag=f"acc{cb}")
                                for kk in range(NKK):
                                    nc.tensor.matmul(
                                        acc[:],
                                        hT[:, kk, tt * P : (tt + 1) * P],
                                        wT[:, kk, cb * 512 : (cb + 1) * 512],
                                        start=(kk == 0), stop=(kk == NKK - 1))
                                dst = qkv_bf[:, ti_e, cb * 512 : (cb + 1) * 512]
                                if kh == 0:
                                    nc.any.tensor_copy(dst, acc[:])
                                else:
                                    nc.vector.tensor_add(dst, dst, acc[:])
                        if kh == 0:
                            continue
                        # ---- LN + affine + rope + transpose (full qkv now) ----
                        for tt in range(TG // P):
                            ta = t0 + tt * P  # absolute t-tile start
                            ti = ta // P
                            nc.any.tensor_copy(v_nat[:, ti, :],
                                               qkv_bf[:, ti, 2 * 512 : 3 * 512])
                            for grp in range(2):  # 0: q heads, 1: k heads
                                xg = qkv_bf[:, ti, grp * 512 : (grp + 1) * 512]
                                xg4 = xg.rearrange("p (h d) -> p h d", h=HPC)
                                s1 = p_st.tile([P, HPC], f32, tag="s1")
                                s2 = p_st.tile([P, HPC], f32, tag="s2")
                                sqs = p_scr.tile([P, 512], bf16, tag="sqs")
                                for h in range(HPC):
                                    nc.scalar.activation(
                                        sqs[:, h * D : (h + 1) * D],
                                        xg4[:, h, :], AF.Square,
                                        accum_out=s2[:, h : h + 1])
                                nc.vector.tensor_reduce(
                                    s1[:], xg4, axis=mybir.AxisListType.X,
                                    op=ALU.add)
                                mu = p_st.tile([P, HPC], f32, tag="mu")
                                nc.vector.tensor_scalar_mul(mu[:], s1[:], 1.0 / D)
                                musq = p_st.tile([P, HPC], f32, tag="musq")
                                nc.vector.tensor_mul(musq[:], mu[:], mu[:])
                                varv = p_st.tile([P, HPC], f32, tag="varv")
                                nc.vector.scalar_tensor_tensor(
                                    out=varv[:], in0=s2[:], scalar=1.0 / D,
                                    in1=musq[:], op0=ALU.mult, op1=ALU.subtract)
                                stdv = p_st.tile([P, HPC], f32, tag="stdv")
                                nc.scalar.activation(stdv[:], varv[:], AF.Sqrt,
                                                     bias=epsc[:])
                                rstd = p_st.tile([P, HPC], f32, tag="rstd")
                                nc.vector.reciprocal(rstd[:], stdv[:])
                                y = p_scr.tile([P, 512], bf16, tag="y")
                                for h in range(HPC):
                                    nc.vector.tensor_scalar(
                                        out=y[:, h * D : (h + 1) * D],
                                        in0=xg4[:, h, :],
                                        scalar1=mu[:, h : h + 1],
                                        scalar2=rstd[:, h : h + 1],
                                        op0=ALU.subtract, op1=ALU.mult)
                                y4 = y.rearrange("p (h d) -> p h d", h=HPC)
                                nc.vector.tensor_mul(
                                    y4, y4, w8[:, grp * 4 : grp * 4 + 4, :])
                                nc.vector.tensor_add(
                                    y4, y4, b8[:, grp * 4 : grp * 4 + 4, :])
                                # rope: halves along d
                                yh = y.rearrange("p (h v d) -> p h v d", h=HPC, v=2)
                                ro = p_scr.tile([P, 512], bf16, tag="ro")
                                roh = ro.rearrange("p (h v d) -> p h v d", h=HPC, v=2)
                                tmp = p_scr.tile([P, 512], bf16, tag="tmp")
                                tmph = tmp.rearrange("p (h v d) -> p h v d", h=HPC, v=2)
                                cc = c4[:, ti, :, :]
                                ss = s4[:, ti, :, :]
                                nc.vector.tensor_mul(tmph[:, :, 0, :], yh[:, :, 0, :], cc)
                                nc.vector.tensor_mul(tmph[:, :, 1, :], yh[:, :, 1, :], ss)
                                nc.vector.tensor_sub(roh[:, :, 0, :], tmph[:, :, 0, :],
                                                     tmph[:, :, 1, :])
                                nc.vector.tensor_mul(tmph[:, :, 0, :], yh[:, :, 1, :], cc)
                                nc.vector.tensor_mul(tmph[:, :, 1, :], yh[:, :, 0, :], ss)
                                nc.vector.tensor_add(roh[:, :, 1, :], tmph[:, :, 0, :],
                                                     tmph[:, :, 1, :])
                                # transpose 4 heads -> qkT[d, grp*4+h, t]
                                pq = ps_tp.tile([P, 4, P], bf16, tag="tpb")
                                for h in range(HPC):
                                    nc.tensor.transpose(
                                        pq[:, h, :], ro[:, h * D : (h + 1) * D],
                                        ident_b[:])
                                nc.any.tensor_copy(
                                    qkT[:, grp * 4 : grp * 4 + 4, ta : ta + P],
                                    pq[:])

            if dump:
                nc.sync.dma_start(d_qkT[:], qkT[:])
                nc.sync.dma_start(d_vnat[:], v_nat[:])

            # ---------------- P2: causal attention ----------------
            with tc.tile_pool(name="p2_attnT", bufs=1) as p_attnT:
                attnT = p_attnT.tile([P, HPC, T], bf16, tag="attnT")
                with (
                    tc.tile_pool(name="p2_E", bufs=4) as p_E,
                    tc.tile_pool(name="p2_tail", bufs=2) as p_tail,
                    tc.tile_pool(name="ps_st", bufs=3, space="PSUM") as ps_st,
                    tc.tile_pool(name="ps_at", bufs=2, space="PSUM") as ps_at,
                ):
                    for h in range(HPC):
                        for b in range(4):
                            bs = slice(b * 512, (b + 1) * 512)
                            at_ps = ps_at.tile([P, 512], f32, tag="at")
                            rsum = ps_at.tile([1, 512], f32, tag="rsum")
                            n_i = 4 * b + 4
                            for i in range(n_i):
                                moff = max(0, (i - 4 * b) * P)
                                nv = 512 - moff
                                st = ps_st.tile([P, 512], f32, tag="st")
                                nc.tensor.matmul(
                                    st[:, 0:nv],
                                    qkT[:, 4 + h, i * P : (i + 1) * P],
                                    qkT[:, h, b * 512 + moff : (b + 1) * 512],
                                    start=True, stop=True)
                                E = p_E.tile([P, 512], bf16, tag="E")
                                if moff:
                                    nc.vector.memset(E[:, 0:moff], 0.0)
                                nc.scalar.activation(E[:, moff:512], st[:, 0:nv],
                                                     AF.Exp, scale=SCALE)
                                if i >= 4 * b:
                                    nc.vector.tensor_mul(
                                        E[:, moff : moff + P],
                                        E[:, moff : moff + P], triu_b[:])
                                nc.tensor.matmul(
                                    at_ps[:], v_nat[:, i, h * D : (h + 1) * D],
                                    E[:], start=(i == 0), stop=(i == n_i - 1))
                                nc.tensor.matmul(
                                    rsum[:], ones_c[:], E[:],
                                    start=(i == 0), stop=(i == n_i - 1))
                            rs_sb = p_tail.tile([1, 512], f32, tag="rs_sb")
                            nc.scalar.activation(rs_sb[:], rsum[:], AF.Copy)
                            rc = p_tail.tile([1, 512], f32, tag="rc")
                            nc.vector.reciprocal_approx_fast(out=rc[:], in_=rs_sb[:])
                            recb = p_tail.tile([P, 512], f32, tag="recb")
                            nc.gpsimd.partition_broadcast(recb[:], rc[:])
                            nc.vector.tensor_mul(attnT[:, h, bs], at_ps[:], recb[:])

                if dump:
                    nc.sync.dma_start(d_attnT[:], attnT[:])

                # ---------------- P3: o_proj ----------------
                with (
                    tc.tile_pool(name="p3_wT", bufs=2) as p3_wT,
                    tc.tile_pool(name="p3_o", bufs=3) as p3_o,
                    tc.tile_pool(name="ps_o", bufs=2, space="PSUM") as ps_o,
                ):
                    for nb in range(HID // 512):
                        wot = p3_wT.tile([P, 4, 512], bf16, tag="woT")
                        nc.sync.dma_start(
                            wot[:],
                            woT[:, nb * 512 : (nb + 1) * 512]
                            .rearrange("(c p) n -> p c n", p=P))
                        for tg4 in range(T // 512):
                            o_sb = p3_o.tile([P, 4, 512], bf16, tag="o_sb")
                            for j in range(4):
                                t = tg4 * 4 + j
                                po = ps_o.tile([P, 512], f32, tag="po")
                                for c in range(4):
                                    nc.tensor.matmul(
                                        po[:],
                                        attnT[:, c, t * P : (t + 1) * P],
                                        wot[:, c, :],
                                        start=(c == 0), stop=(c == 3))
                                nc.any.tensor_copy(o_sb[:, j, :], po[:])
                            nc.sync.dma_start(
                                out[tg4 * 512 : (tg4 + 1) * 512,
                                    nb * 512 : (nb + 1) * 512]
                                .rearrange("(a p) n -> p a n", p=P),
                                o_sb[:])

    nc.compile()
    return nc


def _get_nc():
    if "nc" not in _NC_CACHE:
        _NC_CACHE["nc"] = build_nc()
    return _NC_CACHE["nc"]


def _shard_inputs(positions, hidden_states, w_qkv, w_o, q_norm_w, q_norm_b,
                  k_norm_w, k_norm_b):
    H = 32
    bf = ml_dtypes.bfloat16
    hT = np.ascontiguousarray(np.asarray(hidden_states, np.float32).T.astype(bf))
    pos_np = np.ascontiguousarray(positions, dtype=np.int32)
    in_maps = []
    for c in range(8):
        hs = slice(c * HPC, (c + 1) * HPC)
        rows = np.concatenate(
            [
                w_qkv[c * 512 : (c + 1) * 512],
                w_qkv[H * D + c * 512 : H * D + (c + 1) * 512],
                w_qkv[2 * H * D + c * 512 : 2 * H * D + (c + 1) * 512],
            ],
            axis=0,
        )
        in_maps.append(
            {
                "hiddenT": hT,
                "wqT": np.ascontiguousarray(
                    np.asarray(rows, np.float32).T.astype(bf)),
                "woT": np.ascontiguousarray(
                    np.asarray(w_o[:, c * 512 : (c + 1) * 512], np.float32)
                    .T.astype(bf)),
                "pos": pos_np,
                "qnw": np.ascontiguousarray(q_norm_w[hs], dtype=np.float32),
                "qnb": np.ascontiguousarray(q_norm_b[hs], dtype=np.float32),
                "knw": np.ascontiguousarray(k_norm_w[hs], dtype=np.float32),
                "knb": np.ascontiguousarray(k_norm_b[hs], dtype=np.float32),
            }
        )
    return in_maps


def kernel(positions, hidden_states, w_qkv, w_o, q_norm_w, q_norm_b,
           k_norm_w, k_norm_b):
    nc = _get_nc()
    in_maps = _shard_inputs(
        np.asarray(positions), np.asarray(hidden_states), np.asarray(w_qkv),
        np.asarray(w_o), np.asarray(q_norm_w), np.asarray(q_norm_b),
        np.asarray(k_norm_w), np.asarray(k_norm_b),
    )
    res = run_bass_kernel_spmd(nc, in_maps, list(range(8))).results
    acc = np.zeros((T, HID), np.float64)
    for c in range(8):
        acc += res[c]["out"].astype(np.float64)
    return acc.astype(np.float32)


if __name__ == "__main__":
    build_nc()
    print("build OK")


# revision 10
# speedup vs baseline: 2.0084x; 1.0229x over previous
"""ChameleonAttention Trainium2 kernel (v4).

Full-input contract: kernel(**inputs) with the complete tensors; internally
shards tensor-parallel across 8 NeuronCores by attention head (4 heads/core):
  - w_qkv rows + q/k norm params sharded by head
  - w_o columns sharded by head, partial outputs summed on host (all-reduce)

v4: host pre-transposes and pre-converts the big operands to bf16
(hidden^T, w_qkv^T, w_o^T) so tiles DMA straight into matmul-ready layouts
(zero weight/activation transposes on device). Single full-K P1 pass
(wT fully resident, 128-token groups, PSUM accumulation over all 32
k-tiles), so qkv needs no partial accumulator. P3 is interleaved with
attention (q-block-outer loop, w_o fully resident) so o_proj matmuls fill
the softmax-latency gaps and the output DMA streams throughout.

Per-core dataflow:
  P1 per 128-token group: qkv_nat[t,1536] = hT.T @ wT (3x N=512 chains,
      stationary=hT tile); then LN (ACT Square+accum / DVE reduce,
      per-partition stats), affine, neox RoPE (free-dim halves, Cody-Waite
      sin/cos), PE-transpose q,k into qkT[d,t]; v copied natural.
  P2/P3 per q-block b: 4 heads of causal attention (S^T on PE, exp on ACT,
      bf16 E, triu mask on DVE, P@V + ones-rsum in PSUM,
      reciprocal_approx_fast denominator), then o_proj rows for block b.
"""
import sys

sys.path.insert(0, "/opt/trn_rl_repo")

import numpy as np
import ml_dtypes

import concourse.bass as bass
import concourse.mybir as mybir
import concourse.tile as tile
from concourse import bacc
from concourse.bass_utils import run_bass_kernel_spmd
from concourse.masks import make_identity, make_upper_triangular

P = 128
T = 2048
HID = 4096
D = 128
HPC = 4  # heads per core
R = 3 * HPC  # 12 qkv row-tiles per core
NK = HID // P  # 32 k-tiles
NTT = T // P  # 16 token tiles
THETA = 10000.0
EPS = 1e-5
SCALE = D ** -0.5
TWO_PI = 6.283185307179586
C_HI = float(np.float32(6.28125))
C_LO = TWO_PI - C_HI

f32 = mybir.dt.float32
bf16 = mybir.dt.bfloat16
i32 = mybir.dt.int32
AF = mybir.ActivationFunctionType
ALU = mybir.AluOpType

_NC_CACHE = {}


def build_nc(dump: bool = False):
    nc = bacc.Bacc(None, target_bir_lowering=False, debug=False)

    hiddenT = nc.dram_tensor("hiddenT", (HID, T), bf16, kind="ExternalInput")
    wqT = nc.dram_tensor("wqT", (HID, R * P), bf16, kind="ExternalInput")
    woT = nc.dram_tensor("woT", (HPC * D, HID), bf16, kind="ExternalInput")
    pos = nc.dram_tensor("pos", (T,), i32, kind="ExternalInput")
    qnw = nc.dram_tensor("qnw", (HPC, D), f32, kind="ExternalInput")
    qnb = nc.dram_tensor("qnb", (HPC, D), f32, kind="ExternalInput")
    knw = nc.dram_tensor("knw", (HPC, D), f32, kind="ExternalInput")
    knb = nc.dram_tensor("knb", (HPC, D), f32, kind="ExternalInput")
    out = nc.dram_tensor("out", (T, HID), bf16, kind="ExternalOutput")
    if dump:
        d_qkT = nc.dram_tensor("d_qkT", (P, 8, T), bf16, kind="ExternalOutput")
        d_vnat = nc.dram_tensor("d_vnat", (P, NTT, 512), bf16, kind="ExternalOutput")
        d_attnT = nc.dram_tensor("d_attnT", (P, HPC, T), bf16, kind="ExternalOutput")

    with tile.TileContext(nc) as tc:
        with tc.tile_pool(name="const", bufs=1) as const:
            p1_scope = tc.tile_pool(name="p1_wT", bufs=1)
            p_wT = p1_scope.__enter__()
            p1_scope_h = tc.tile_pool(name="p1_hT", bufs=2)
            p_hT = p1_scope_h.__enter__()
            # --- kick off the big DMAs first so PE can start ASAP ---
            wT = p_wT.tile([P, NK, R * P], bf16, tag="wT")
            wsrc = wqT.rearrange("(kk p) c -> p kk c", p=P)
            for kg in range(8):
                nc.sync.dma_start(wT[:, kg * 4 : (kg + 1) * 4, :],
                                  wsrc[:, kg * 4 : (kg + 1) * 4, :])
            hsrc = hiddenT.rearrange("(kk p) t -> p kk t", p=P)
            hTs = []
            for tg in range(2):  # prefetch first two token tiles
                hT = p_hT.tile([P, NK, P], bf16, tag="hT")
                nc.sync.dma_start(hT[:], hsrc[:, :, tg * P : (tg + 1) * P])
                hTs.append(hT)

            # --- constants ---
            ident_b = const.tile([P, P], bf16)
            triu_b = const.tile([P, P], bf16)
            ones_c = const.tile([P, 1], bf16)
            epsc = const.tile([P, 1], f32)
            nc.vector.memset(epsc[:], EPS)
            w8 = const.tile([P, 8, D], bf16)  # LN weight, bcast over t
            b8 = const.tile([P, 8, D], bf16)
            c4 = const.tile([P, NTT, HPC, 64], bf16)  # cos, replicated x4 heads
            s4 = const.tile([P, NTT, HPC, 64], bf16)
            qkT = const.tile([P, 8, T], bf16)  # post-rope q(0:4), k(4:8); [d, t]
            v_nat = const.tile([P, NTT, 512], bf16)  # [t, 4 heads * d]

            with tc.tile_pool(name="cstage", bufs=1) as cstage:
                ident_f = cstage.tile([P, P], f32)
                make_identity(nc, ident_f[:])
                nc.vector.tensor_copy(ident_b[:], ident_f[:])
                triu_f = cstage.tile([P, P], f32)
                make_upper_triangular(nc, triu_f[:], val=1.0, diag=True)
                nc.vector.tensor_copy(triu_b[:], triu_f[:])
                ones_f = cstage.tile([P, 1], f32)
                nc.vector.memset(ones_f[:], 1.0)
                nc.vector.tensor_copy(ones_c[:], ones_f[:])

                # LN affine params broadcast over partitions: w8/b8[p, g*4+h, :]
                nrm = cstage.tile([1, 8, D], f32)
                for h in range(HPC):
                    nc.sync.dma_start(nrm[:, h, :], qnw[h : h + 1, :])
                    nc.sync.dma_start(nrm[:, 4 + h, :], knw[h : h + 1, :])
                nrm_b = cstage.tile([P, 8, D], f32)
                nc.gpsimd.partition_broadcast(nrm_b[:], nrm[:])
                nc.vector.tensor_copy(w8[:], nrm_b[:])
                nrm2 = cstage.tile([1, 8, D], f32, tag="nrm2")
                for h in range(HPC):
                    nc.sync.dma_start(nrm2[:, h, :], qnb[h : h + 1, :])
                    nc.sync.dma_start(nrm2[:, 4 + h, :], knb[h : h + 1, :])
                nrm2_b = cstage.tile([P, 8, D], f32, tag="nrm2b")
                nc.gpsimd.partition_broadcast(nrm2_b[:], nrm2[:])
                nc.vector.tensor_copy(b8[:], nrm2_b[:])

            # --- rope tables in natural layout: [t(part), tt, 64] ---
            with tc.tile_pool(name="rtmp", bufs=1) as rtmp:
                jj = rtmp.tile([1, 64], f32)
                nc.gpsimd.iota(jj[:], pattern=[[1, 64]], base=0,
                               channel_multiplier=0,
                               allow_small_or_imprecise_dtypes=True)
                invf = rtmp.tile([1, 64], f32)
                nc.scalar.activation(invf[:], jj[:], AF.Exp,
                                     scale=-float(np.log(THETA)) / 64.0)
                invf_b = rtmp.tile([P, 64], f32)
                nc.gpsimd.partition_broadcast(invf_b[:], invf[:])
                # t values from positions input: tval[p, i] = pos[i*128 + p]
                pos_i = rtmp.tile([P, NTT], i32)
                nc.sync.dma_start(pos_i[:], pos.rearrange("(i p) -> p i", p=P))
                tval = rtmp.tile([P, NTT], f32)
                nc.vector.tensor_copy(tval[:], pos_i[:])
                freqs = rtmp.tile([P, NTT, 64], f32)
                for i in range(NTT):
                    nc.vector.tensor_scalar_mul(freqs[:, i, :], invf_b[:],
                                                tval[:, i : i + 1])

                HT2 = NTT // 2

                def reduced_sin(dst_ap, src_ap):
                    # dst = sin(reduce(src)), reduce(x) = x - 2pi*round(x/2pi)
                    q = rtmp.tile([P, HT2, 64], f32, tag="rs_q")
                    nc.vector.tensor_scalar_mul(q[:], src_ap, 1.0 / TWO_PI)
                    n_i = rtmp.tile([P, HT2, 64], i32, tag="rs_n")
                    nc.vector.tensor_copy(n_i[:], q[:])  # round-to-nearest
                    n_f = rtmp.tile([P, HT2, 64], f32, tag="rs_nf")
                    nc.vector.tensor_copy(n_f[:], n_i[:])
                    r0 = rtmp.tile([P, HT2, 64], f32, tag="rs_r0")
                    nc.vector.scalar_tensor_tensor(
                        out=r0[:], in0=n_f[:], scalar=-C_HI, in1=src_ap,
                        op0=ALU.mult, op1=ALU.add)
                    r1 = rtmp.tile([P, HT2, 64], f32, tag="rs_r1")
                    nc.vector.scalar_tensor_tensor(
                        out=r1[:], in0=n_f[:], scalar=-C_LO, in1=r0[:],
                        op0=ALU.mult, op1=ALU.add)
                    nc.scalar.activation(dst_ap, r1[:], AF.Sin)

                for hh in range(2):
                    tsl = slice(hh * HT2, (hh + 1) * HT2)
                    sc1 = rtmp.tile([P, HT2, 64], bf16, tag="sc1")
                    reduced_sin(sc1[:], freqs[:, tsl, :])
                    for h in range(HPC):
                        nc.vector.tensor_copy(s4[:, tsl, h, :], sc1[:])
                    fr2 = rtmp.tile([P, HT2, 64], f32, tag="fr2")
                    nc.vector.tensor_scalar_add(fr2[:], freqs[:, tsl, :],
                                                np.pi / 2)
                    sc2 = rtmp.tile([P, HT2, 64], bf16, tag="sc2")
                    reduced_sin(sc2[:], fr2[:])
                    for h in range(HPC):
                        nc.vector.tensor_copy(c4[:, tsl, h, :], sc2[:])

            # ---------------- P1 + LN + RoPE (per 128-token tile) ----------
            with (
                tc.tile_pool(name="p1_qkv", bufs=2) as p_qkv,
                tc.tile_pool(name="p1_scr", bufs=2) as p_scr,
                tc.tile_pool(name="p1_st", bufs=2) as p_st,
                tc.tile_pool(name="ps_tp", bufs=2, space="PSUM") as ps_tp,
                tc.tile_pool(name="ps_acc", bufs=2, space="PSUM") as ps_acc,
            ):
                for tg in range(NTT):
                    ta = tg * P
                    if tg < 2:
                        hT = hTs[tg]
                    else:
                        hT = p_hT.tile([P, NK, P], bf16, tag="hT")
                        nc.sync.dma_start(hT[:], hsrc[:, :, ta : ta + P])
                    qkv = p_qkv.tile([P, R * P], bf16, tag="qkv")
                    for cb in range(3):
                        acc = ps_acc.tile([P, 512], f32, tag=f"acc{cb}")
                        for kk in range(NK):
                            nc.tensor.matmul(
                                acc[:],
                                hT[:, kk, :],
                                wT[:, kk, cb * 512 : (cb + 1) * 512],
                                start=(kk == 0), stop=(kk == NK - 1))
                        nc.any.tensor_copy(qkv[:, cb * 512 : (cb + 1) * 512],
                                           acc[:])
                    # ---- LN + affine + rope + transpose ----
                    nc.any.tensor_copy(v_nat[:, tg, :], qkv[:, 2 * 512 : 3 * 512])
                    for grp in range(2):  # 0: q heads, 1: k heads
                        xg = qkv[:, grp * 512 : (grp + 1) * 512]
                        xg4 = xg.rearrange("p (h d) -> p h d", h=HPC)
                        s1 = p_st.tile([P, HPC], f32, tag="s1")
                        s2 = p_st.tile([P, HPC], f32, tag="s2")
                        sqs = p_scr.tile([P, 512], bf16, tag="sqs")
                        for h in range(HPC):
                            nc.scalar.activation(
                                sqs[:, h * D : (h + 1) * D],
                                xg4[:, h, :], AF.Square,
                                accum_out=s2[:, h : h + 1])
                        nc.vector.tensor_reduce(
                            s1[:], xg4, axis=mybir.AxisListType.X, op=ALU.add)
                        mu = p_st.tile([P, HPC], f32, tag="mu")
                        nc.vector.tensor_scalar_mul(mu[:], s1[:], 1.0 / D)
                        musq = p_st.tile([P, HPC], f32, tag="musq")
                        nc.vector.tensor_mul(musq[:], mu[:], mu[:])
                        varv = p_st.tile([P, HPC], f32, tag="varv")
                        nc.vector.scalar_tensor_tensor(
                            out=varv[:], in0=s2[:], scalar=1.0 / D,
                            in1=musq[:], op0=ALU.mult, op1=ALU.subtract)
                        stdv = p_st.tile([P, HPC], f32, tag="stdv")
                        nc.scalar.activation(stdv[:], varv[:], AF.Sqrt,
                                             bias=epsc[:])
                        rstd = p_st.tile([P, HPC], f32, tag="rstd")
                        nc.vector.reciprocal(rstd[:], stdv[:])
                        y = p_scr.tile([P, 512], bf16, tag="y")
                        for h in range(HPC):
                            nc.vector.tensor_scalar(
                                out=y[:, h * D : (h + 1) * D],
                                in0=xg4[:, h, :],
                                scalar1=mu[:, h : h + 1],
                                scalar2=rstd[:, h : h + 1],
                                op0=ALU.subtract, op1=ALU.mult)
                        y4 = y.rearrange("p (h d) -> p h d", h=HPC)
                        nc.vector.tensor_mul(y4, y4, w8[:, grp * 4 : grp * 4 + 4, :])
                        nc.vector.tensor_add(y4, y4, b8[:, grp * 4 : grp * 4 + 4, :])
                        # rope: halves along d
                        yh = y.rearrange("p (h v d) -> p h v d", h=HPC, v=2)
                        ro = p_scr.tile([P, 512], bf16, tag="ro")
                        roh = ro.rearrange("p (h v d) -> p h v d", h=HPC, v=2)
                        tmp = p_scr.tile([P, 512], bf16, tag="tmp")
                        tmph = tmp.rearrange("p (h v d) -> p h v d", h=HPC, v=2)
                        cc = c4[:, tg, :, :]
                        ss = s4[:, tg, :, :]
                        nc.vector.tensor_mul(tmph[:, :, 0, :], yh[:, :, 0, :], cc)
                        nc.vector.tensor_mul(tmph[:, :, 1, :], yh[:, :, 1, :], ss)
                        nc.vector.tensor_sub(roh[:, :, 0, :], tmph[:, :, 0, :],
                                             tmph[:, :, 1, :])
                        nc.vector.tensor_mul(tmph[:, :, 0, :], yh[:, :, 1, :], cc)
                        nc.vector.tensor_mul(tmph[:, :, 1, :], yh[:, :, 0, :], ss)
                        nc.vector.tensor_add(roh[:, :, 1, :], tmph[:, :, 0, :],
                                             tmph[:, :, 1, :])
                        # transpose 4 heads -> qkT[d, grp*4+h, t]
                        pq = ps_tp.tile([P, 4, P], bf16, tag="tpb")
                        for h in range(HPC):
                            nc.tensor.transpose(
                                pq[:, h, :], ro[:, h * D : (h + 1) * D],
                                ident_b[:])
                        nc.any.tensor_copy(
                            qkT[:, grp * 4 : grp * 4 + 4, ta : ta + P], pq[:])

            p1_scope_h.__exit__(None, None, None)
            p1_scope.__exit__(None, None, None)

            if dump:
                nc.sync.dma_start(d_qkT[:], qkT[:])
                nc.sync.dma_start(d_vnat[:], v_nat[:])

            # -------- P2 + P3 interleaved per q-block --------
            with (
                tc.tile_pool(name="p2_attnT", bufs=1) as p_attnT,
                tc.tile_pool(name="p3_wo", bufs=1) as p3_wo,
            ):
                attnT = p_attnT.tile([P, HPC, T], bf16, tag="attnT")
                wo_all = p3_wo.tile([P, 8, 4, 512], bf16, tag="wo_all")
                wos = woT.rearrange("(c p) (nb n) -> p nb c n", p=P, n=512)
                for nb in range(8):
                    nc.sync.dma_start(wo_all[:, nb, :, :], wos[:, nb, :, :])
                with (
                    tc.tile_pool(name="p2_E", bufs=4) as p_E,
                    tc.tile_pool(name="p2_tail", bufs=2) as p_tail,
                    tc.tile_pool(name="p3_o", bufs=3) as p3_o,
                    tc.tile_pool(name="ps_st", bufs=2, space="PSUM") as ps_st,
                    tc.tile_pool(name="ps_at", bufs=2, space="PSUM") as ps_at,
                    tc.tile_pool(name="ps_o", bufs=2, space="PSUM") as ps_o,
                ):
                    for b in range(4):
                        bs = slice(b * 512, (b + 1) * 512)
                        for h in range(HPC):
                            at_ps = ps_at.tile([P, 512], f32, tag="at")
                            rsum = ps_at.tile([1, 512], f32, tag="rsum")
                            n_i = 4 * b + 4
                            for i in range(n_i):
                                moff = max(0, (i - 4 * b) * P)
                                nv = 512 - moff
                                st = ps_st.tile([P, 512], f32, tag="st")
                                nc.tensor.matmul(
                                    st[:, 0:nv],
                                    qkT[:, 4 + h, i * P : (i + 1) * P],
                                    qkT[:, h, b * 512 + moff : (b + 1) * 512],
                                    start=True, stop=True)
                                E = p_E.tile([P, 512], bf16, tag="E")
                                if moff:
                                    nc.vector.memset(E[:, 0:moff], 0.0)
                                nc.scalar.activation(E[:, moff:512], st[:, 0:nv],
                                                     AF.Exp, scale=SCALE)
                                if i >= 4 * b:
                                    nc.vector.tensor_mul(
                                        E[:, moff : moff + P],
                                        E[:, moff : moff + P], triu_b[:])
                                nc.tensor.matmul(
                                    at_ps[:], v_nat[:, i, h * D : (h + 1) * D],
                                    E[:], start=(i == 0), stop=(i == n_i - 1))
                                nc.tensor.matmul(
                                    rsum[:], ones_c[:], E[:],
                                    start=(i == 0), stop=(i == n_i - 1))
                            rs_sb = p_tail.tile([1, 512], f32, tag="rs_sb")
                            nc.scalar.activation(rs_sb[:], rsum[:], AF.Copy)
                            rc = p_tail.tile([1, 512], f32, tag="rc")
                            nc.vector.reciprocal_approx_fast(out=rc[:], in_=rs_sb[:])
                            recb = p_tail.tile([P, 512], f32, tag="recb")
                            nc.gpsimd.partition_broadcast(recb[:], rc[:])
                            nc.vector.tensor_mul(attnT[:, h, bs], at_ps[:], recb[:])
                        # ---- o_proj rows for this q-block ----
                        for nb in range(8):
                            o_sb = p3_o.tile([P, 4, 512], bf16, tag="o_sb")
                            for j in range(4):
                                t = 4 * b + j
                                po = ps_o.tile([P, 512], f32, tag="po")
                                for c in range(4):
                                    nc.tensor.matmul(
                                        po[:],
                                        attnT[:, c, t * P : (t + 1) * P],
                                        wo_all[:, nb, c, :],
                                        start=(c == 0), stop=(c == 3))
                                nc.any.tensor_copy(o_sb[:, j, :], po[:])
                            nc.sync.dma_start(
                                out[b * 512 : (b + 1) * 512,
                                    nb * 512 : (nb + 1) * 512]
                                .rearrange("(a p) n -> p a n", p=P),
                                o_sb[:])

                if dump:
                    nc.sync.dma_start(d_attnT[:], attnT[:])

    nc.compile()
    return nc


def _get_nc():
    if "nc" not in _NC_CACHE:
        _NC_CACHE["nc"] = build_nc()
    return _NC_CACHE["nc"]


def _shard_inputs(positions, hidden_states, w_qkv, w_o, q_norm_w, q_norm_b,
                  k_norm_w, k_norm_b):
    H = 32
    bf = ml_dtypes.bfloat16
    hT = np.ascontiguousarray(np.asarray(hidden_states, np.float32).T.astype(bf))
    pos_np = np.ascontiguousarray(positions, dtype=np.int32)
    in_maps = []
    for c in range(8):
        hs = slice(c * HPC, (c + 1) * HPC)
        rows = np.concatenate(
            [
                w_qkv[c * 512 : (c + 1) * 512],
                w_qkv[H * D + c * 512 : H * D + (c + 1) * 512],
                w_qkv[2 * H * D + c * 512 : 2 * H * D + (c + 1) * 512],
            ],
            axis=0,
        )
        in_maps.append(
            {
                "hiddenT": hT,
                "wqT": np.ascontiguousarray(
                    np.asarray(rows, np.float32).T.astype(bf)),
                "woT": np.ascontiguousarray(
                    np.asarray(w_o[:, c * 512 : (c + 1) * 512], np.float32)
                    .T.astype(bf)),
                "pos": pos_np,
                "qnw": np.ascontiguousarray(q_norm_w[hs], dtype=np.float32),
                "qnb": np.ascontiguousarray(q_norm_b[hs], dtype=np.float32),
                "knw": np.ascontiguousarray(k_norm_w[hs], dtype=np.float32),
                "knb": np.ascontiguousarray(k_norm_b[hs], dtype=np.float32),
            }
        )
    return in_maps


def kernel(positions, hidden_states, w_qkv, w_o, q_norm_w, q_norm_b,
           k_norm_w, k_norm_b):
    nc = _get_nc()
    in_maps = _shard_inputs(
        np.asarray(positions), np.asarray(hidden_states), np.asarray(w_qkv),
        np.asarray(w_o), np.asarray(q_norm_w), np.asarray(q_norm_b),
        np.asarray(k_norm_w), np.asarray(k_norm_b),
    )
    res = run_bass_kernel_spmd(nc, in_maps, list(range(8))).results
    acc = np.zeros((T, HID), np.float64)
    for c in range(8):
        acc += res[c]["out"].astype(np.float64)
    return acc.astype(np.float32)


if __name__ == "__main__":
    build_nc()
    print("build OK")
